# revision 1
# baseline (speedup 1.0000x reference)
"""CTBG circuit kernel for Trainium2, data-parallel over batch on 8 NeuronCores.

Network (per reference):
  gpe_out = x @ (gpe_w * gpe_mask.T) + gpe_b              [B, 1536]
  gpi_in  = concat([x, gpe_out], -1)                      [B, 3072]
  gpi_out = gpi_in @ (gpi_w * gpi_mask.T) + gpi_b         [B, 1536]
  h1 = relu(gpi_out @ w1 + b1); h2 = relu(h1 @ w2 + b2)
  out = relu(h2 @ w3 + b3)                                [B, 6]

Per-core dataflow (feature-major activations, bf16 compute, f32 accumulate):
  - NO DMA-xbar transposes (transpose<->copy xbar-mode transitions serialize
    the whole DMA subsystem on trn2); every transpose runs on the
    TensorEngine via identity matmuls instead, overlapped with loads.
  - x and masks stream in as bf16 row-tiles (SWDGE cast-DMA straight to
    SBUF, no DRAM staging); weights stream as f32 rows on the scalar HWDGE
    queue, cast to resident bf16 tiles by DVE.
  - masked weights: PE-transpose each 128x128 mask block into PSUM, then
    DVE multiplies it into the resident bf16 weight tile in place.
  - matmul chain keeps activations feature-major; ScalarE drains PSUM with
    bias (+relu for the MLP) straight to bf16 tiles feeding the next layer.
  - output written as [6, 2048] f32 per core; host transposes + concats.
"""

import numpy as np

NCORES = 8
B = 16384
BS = B // NCORES          # 2048 rows per core
BT = 512                  # batch tile (matmul free dim)
NBT = BS // BT            # 4
D1 = 1536                 # gpe input dim
D2 = 1536                 # gpe output dim
D3 = 3072                 # gpi input dim
D4 = 1536                 # gpi output dim
H = 512                   # mlp hidden
A = 6                     # action dim

K1 = D1 // 128            # 12
U2 = D2 // 128            # 12
K3 = D3 // 128            # 24
V4 = D4 // 128            # 12
M5 = H // 128             # 4

_CACHE = {}


def _build():
    import concourse.bacc as bacc
    import concourse.tile as tile
    from concourse import mybir
    from concourse.masks import make_identity

    FP32 = mybir.dt.float32
    BF16 = mybir.dt.bfloat16
    Act = mybir.ActivationFunctionType

    nc = bacc.Bacc(None)

    x_d = nc.dram_tensor("x", [BS, D1], FP32, kind="ExternalInput")
    gpem_d = nc.dram_tensor("gpe_mask", [D2, D1], FP32, kind="ExternalInput")
    gpew_d = nc.dram_tensor("gpe_w", [D1, D2], FP32, kind="ExternalInput")
    gpeb_d = nc.dram_tensor("gpe_b", [D2], FP32, kind="ExternalInput")
    gpim_d = nc.dram_tensor("gpi_mask", [D4, D3], FP32, kind="ExternalInput")
    gpiw_d = nc.dram_tensor("gpi_w", [D3, D4], FP32, kind="ExternalInput")
    gpib_d = nc.dram_tensor("gpi_b", [D4], FP32, kind="ExternalInput")
    w1_d = nc.dram_tensor("w1", [D4, H], FP32, kind="ExternalInput")
    b1_d = nc.dram_tensor("b1", [H], FP32, kind="ExternalInput")
    w2_d = nc.dram_tensor("w2", [H, H], FP32, kind="ExternalInput")
    b2_d = nc.dram_tensor("b2", [H], FP32, kind="ExternalInput")
    w3_d = nc.dram_tensor("w3", [H, A], FP32, kind="ExternalInput")
    b3_d = nc.dram_tensor("b3", [A], FP32, kind="ExternalInput")
    o_d = nc.dram_tensor("out", [A, BS], FP32, kind="ExternalOutput")

    with tile.TileContext(nc) as tc:
        with (
            tc.tile_pool(name="wpool", bufs=1) as wp,        # persistent weights
            tc.tile_pool(name="wfpool", bufs=2) as wfp,      # transient f32 weight half-rows
            tc.tile_pool(name="mpool", bufs=3) as mp,        # transient mask row-tiles
            tc.tile_pool(name="xrpool", bufs=2) as xrp,      # transient x row-tiles
            tc.tile_pool(name="xpool", bufs=2) as xp,        # xT double-buffered
            tc.tile_pool(name="apool", bufs=1) as ap,        # activations
            tc.tile_pool(name="opool", bufs=1) as op,        # output staging
            tc.tile_pool(name="pspool", bufs=3, space="PSUM") as psp,
            tc.tile_pool(name="pstpool", bufs=3, space="PSUM") as pstp,
            tc.tile_pool(name="ps5pool", bufs=2, space="PSUM") as ps5p,
        ):
            # ---------------- SWDGE (gpsimd): bf16 cast loads ---------------
            # order = consumption order: x tile0 rows, gpe mask, gpi mask,
            # remaining x rows
            gpem_rows = []
            for u0 in range(U2):
                t = mp.tile([128, D1], BF16, tag="mrow")
                nc.gpsimd.dma_start(out=t[:, :],
                                    in_=gpem_d[u0 * 128:(u0 + 1) * 128, :])
                gpem_rows.append(t)

            xrow0 = []
            for r in range(BT // 128):
                t = xrp.tile([128, D1], BF16, tag="xr")
                nc.gpsimd.dma_start(out=t[:, :], in_=x_d[r * 128:(r + 1) * 128, :])
                xrow0.append(t)

            gpim_rows = []          # (v0, half) -> tile, loaded in v0-major order
            for v0 in range(V4):
                for hh in range(2):
                    t = mp.tile([128, D3 // 2], BF16, tag="mrow")
                    nc.gpsimd.dma_start(
                        out=t[:, :],
                        in_=gpim_d[v0 * 128:(v0 + 1) * 128,
                                   hh * (D3 // 2):(hh + 1) * (D3 // 2)])
                    gpim_rows.append(t)

            xrow_rest = []
            for t_i in range(1, NBT):
                rows = []
                for r in range(BT // 128):
                    g = t_i * (BT // 128) + r
                    t = xrp.tile([128, D1], BF16, tag="xr")
                    nc.gpsimd.dma_start(out=t[:, :],
                                        in_=x_d[g * 128:(g + 1) * 128, :])
                    rows.append(t)
                xrow_rest.append(rows)

            # ---------------- scalar HWDGE: biases + f32 weight halves ------
            ident = wp.tile([128, 128], FP32, tag="ident")
            make_identity(nc, ident[:, :])
            identb = wp.tile([128, 128], BF16, tag="identb")
            make_identity(nc, identb[:, :])

            def load_bias(b_dram, n, tag):
                nat = wp.tile([max(n, 1), 128], FP32, tag=f"{tag}_nat")
                nc.sync.dma_start(out=nat[:, :],
                                    in_=b_dram.rearrange("(c p) -> c p", p=128))
                ps = pstp.tile([128, max(n, 1)], FP32, tag="pst")
                nc.tensor.transpose(ps[:, :], nat[:, :], ident[0:n, 0:n])
                sb = wp.tile([128, max(n, 1)], FP32, tag=tag)
                nc.vector.tensor_copy(sb[:, :], ps[:, :])
                return sb

            gpeb_sb = load_bias(gpeb_d, U2, "gpeb")
            gpib_sb = load_bias(gpib_d, V4, "gpib")
            b1_sb = load_bias(b1_d, M5, "b1sb")
            b2_sb = load_bias(b2_d, M5, "b2sb")
            b3_sb = wp.tile([A, 1], FP32, tag="b3sb")
            nc.sync.dma_start(out=b3_sb[:, :],
                                in_=b3_d.rearrange("(a one) -> a one", one=1))

            def load_w_bf(w_dram, n, width, tag, halves=2):
                """f32 rows on scalar HWDGE (in `halves` column chunks) ->
                DVE cast into a resident bf16 tile."""
                tiles = []
                hw = width // halves
                for k in range(n):
                    t = wp.tile([128, width], BF16, tag=f"{tag}{k}")
                    for hh in range(halves):
                        wf = wfp.tile([128, hw], FP32, tag="wf")
                        nc.sync.dma_start(
                            out=wf[:, 0:hw],
                            in_=w_dram[k * 128:(k + 1) * 128,
                                       hh * hw:(hh + 1) * hw])
                        nc.vector.tensor_copy(t[:, hh * hw:(hh + 1) * hw],
                                              wf[:, 0:hw])
                    tiles.append(t)
                return tiles

            wgpe = load_w_bf(gpew_d, K1, D2, "wgpe")

            # ---------------- PE transposes -------------------------------
            def prep_xT(rows):
                """x row-tiles [128b, D1] -> xT chunk tiles [128i, BT]."""
                tiles = []
                for c in range(K1):
                    t = xp.tile([128, BT], BF16, tag=f"xT{c}")
                    tiles.append(t)
                for r, xrow in enumerate(rows):
                    for c in range(K1):
                        ps = pstp.tile([128, 128], BF16, tag="pst")
                        nc.tensor.transpose(ps[:, :],
                                            xrow[:, c * 128:(c + 1) * 128],
                                            identb[:, :])
                        nc.scalar.activation(
                            tiles[c][:, r * 128:(r + 1) * 128], ps[:, :],
                            mybir.ActivationFunctionType.Copy)
                return tiles

            def prep_mask(rows_for, n_out, n_k, wtiles):
                """PE-transpose mask blocks, DVE-multiply into weight tiles."""
                for o0 in range(n_out):
                    row, col0 = rows_for(o0)
                    # row covers mask[o0*128:(o0+1)*128, col0:col0+ncols]
                    ncols = row.shape[-1]
                    for cc in range(ncols // 128):
                        c = col0 // 128 + cc
                        ps = pstp.tile([128, 128], BF16, tag="pst")
                        nc.tensor.transpose(ps[:, :],
                                            row[:, cc * 128:(cc + 1) * 128],
                                            identb[:, :])
                        nc.vector.tensor_mul(
                            wtiles[c][:, o0 * 128:(o0 + 1) * 128],
                            wtiles[c][:, o0 * 128:(o0 + 1) * 128],
                            ps[:, :])

            prep_mask(lambda u0: (gpem_rows[u0], 0), U2, K1, wgpe)
            xT = prep_xT(xrow0)

            # gpi + mlp weights stream while L1 runs; emitted after the gpe
            # prep so the DVE FIFO (casts) can't head-of-line-block it
            wgpi = load_w_bf(gpiw_d, K3, D4, "wgpi")
            w1s = w2s = w3s = None

            # ---------------- main loop over batch tiles -------------------
            for t_i in range(NBT):
                # L1: gpe_out[u,b] = sum_k mw_gpe[k,u] * xT[k,b]   (+bias)
                gpe_out = []
                for u in range(U2):
                    ps = psp.tile([128, BT], FP32, tag="ps")
                    for k in range(K1):
                        nc.tensor.matmul(ps[:, :],
                                         wgpe[k][:, u * 128:(u + 1) * 128],
                                         xT[k][:, :],
                                         start=(k == 0), stop=(k == K1 - 1))
                    got = ap.tile([128, BT], BF16, tag=f"gpe_out{u}")
                    nc.scalar.activation(got[:, :], ps[:, :], Act.Identity,
                                         bias=gpeb_sb[:, u:u + 1])
                    gpe_out.append(got)
                    if t_i == 0:
                        # gpi masked-weight prep interleaved with L1 so the
                        # PE transposes and DVE muls overlap L1's matmuls
                        for hh in range(2):
                            row = gpim_rows[2 * u + hh]
                            for cc in range(K3 // 2):
                                c = hh * (K3 // 2) + cc
                                pst = pstp.tile([128, 128], BF16, tag="pst")
                                nc.tensor.transpose(
                                    pst[:, :],
                                    row[:, cc * 128:(cc + 1) * 128],
                                    identb[:, :])
                                nc.vector.tensor_mul(
                                    wgpi[c][:, u * 128:(u + 1) * 128],
                                    wgpi[c][:, u * 128:(u + 1) * 128],
                                    pst[:, :])

                if t_i == 0:
                    w1s = load_w_bf(w1_d, V4, H, "w1_", halves=1)
                    w2s = load_w_bf(w2_d, M5, H, "w2_", halves=1)
                    w3s = load_w_bf(w3_d, M5, A, "w3_", halves=1)
                    xT_next = prep_xT(xrow_rest[0])
                elif t_i + 1 < NBT:
                    xT_next = prep_xT(xrow_rest[t_i])
                else:
                    xT_next = None

                # L2: gpi_out[v,b] = sum_k mw_gpi[k,v] * gpi_in[k,b] (+bias)
                gpi_out = []
                for v in range(V4):
                    ps = psp.tile([128, BT], FP32, tag="ps")
                    for k in range(K3):
                        rhs = xT[k] if k < K1 else gpe_out[k - K1]
                        nc.tensor.matmul(ps[:, :],
                                         wgpi[k][:, v * 128:(v + 1) * 128],
                                         rhs[:, :],
                                         start=(k == 0), stop=(k == K3 - 1))
                    gio = ap.tile([128, BT], BF16, tag=f"gpi_out{v}")
                    nc.scalar.activation(gio[:, :], ps[:, :], Act.Identity,
                                         bias=gpib_sb[:, v:v + 1])
                    gpi_out.append(gio)

                # L3: h1 = relu(gpi_out @ w1 + b1)
                h1 = []
                for m in range(M5):
                    ps = psp.tile([128, BT], FP32, tag="ps")
                    for k in range(V4):
                        nc.tensor.matmul(ps[:, :],
                                         w1s[k][:, m * 128:(m + 1) * 128],
                                         gpi_out[k][:, :],
                                         start=(k == 0), stop=(k == V4 - 1))
                    hm = ap.tile([128, BT], BF16, tag=f"h1_{m}")
                    nc.scalar.activation(hm[:, :], ps[:, :], Act.Relu,
                                         bias=b1_sb[:, m:m + 1])
                    h1.append(hm)

                # L4: h2 = relu(h1 @ w2 + b2)
                h2 = []
                for m in range(M5):
                    ps = psp.tile([128, BT], FP32, tag="ps")
                    for k in range(M5):
                        nc.tensor.matmul(ps[:, :],
                                         w2s[k][:, m * 128:(m + 1) * 128],
                                         h1[k][:, :],
                                         start=(k == 0), stop=(k == M5 - 1))
                    hm = ap.tile([128, BT], BF16, tag=f"h2_{m}")
                    nc.scalar.activation(hm[:, :], ps[:, :], Act.Relu,
                                         bias=b2_sb[:, m:m + 1])
                    h2.append(hm)

                # L5: out = relu(h2 @ w3 + b3), [6, BT] f32
                ps5 = ps5p.tile([A, BT], FP32, tag="ps5")
                for k in range(M5):
                    nc.tensor.matmul(ps5[:, :], w3s[k][:, :], h2[k][:, :],
                                     start=(k == 0), stop=(k == M5 - 1))
                osb = op.tile([A, BT], FP32, tag="osb")
                nc.scalar.activation(osb[:, :], ps5[:, :], Act.Relu,
                                     bias=b3_sb[:, 0:1])
                nc.sync.dma_start(out=o_d[:, t_i * BT:(t_i + 1) * BT],
                                    in_=osb[:, :])

                if xT_next is not None:
                    xT = xT_next

    nc.finalize()
    return nc


def _get_nc():
    if "nc" not in _CACHE:
        _CACHE["nc"] = _build()
    return _CACHE["nc"]


def _run(inputs, trace=False):
    from concourse.bass_utils import run_bass_kernel_spmd

    nc = _get_nc()
    shared = {k: np.ascontiguousarray(v, dtype=np.float32)
              for k, v in inputs.items() if k != "x"}
    x = np.ascontiguousarray(inputs["x"], dtype=np.float32)
    in_maps = [dict(shared, x=x[c * BS:(c + 1) * BS]) for c in range(NCORES)]
    res = run_bass_kernel_spmd(nc, in_maps, list(range(NCORES)), trace=trace)
    out = np.concatenate(
        [np.asarray(res.results[c]["out"]).T for c in range(NCORES)], axis=0)
    return out.astype(np.float32), res


def kernel(**inputs):
    out, _ = _run(inputs, trace=False)
    return out



# revision 2
# speedup vs baseline: 2.5343x; 2.5343x over previous
"""CTBG circuit kernel for Trainium2, data-parallel over batch on 8 NeuronCores.

Network (per reference):
  gpe_out = x @ (gpe_w * gpe_mask.T) + gpe_b              [B, 1536]
  gpi_in  = concat([x, gpe_out], -1)                      [B, 3072]
  gpi_out = gpi_in @ (gpi_w * gpi_mask.T) + gpi_b         [B, 1536]
  h1 = relu(gpi_out @ w1 + b1); h2 = relu(h1 @ w2 + b2)
  out = relu(h2 @ w3 + b3)                                [B, 6]

Key algebraic identity: gpe_out and gpi_out feed forward with no
intervening nonlinearity, so the whole masked front end folds into one
[1536, 512] weight computed ON DEVICE once per launch:

  mw_gpe = gpe_w * gpe_mask.T          (masked weights)
  mw_gpi = gpi_w * gpi_mask.T
  M      = mw_gpi[1536:] @ w1                       [1536, 512]
  Wfold  = mw_gpi[:1536] @ w1 + mw_gpe @ M          [1536, 512]
  bfold  = gpe_b @ M + gpi_b @ w1 + b1              [512]
  h1 = relu(x @ Wfold + bfold)   -> h2 -> out       (per batch row)

This cuts per-batch-tile matmul work ~8x; the kernel becomes bound by
streaming the masks/weights from HBM plus the fold matmuls.

Layout/dtype prep happens on host (zero FLOPs): all large tensors are
cast to bf16 and the matrices that the PE needs transposed (x, gpe_w,
gpi_w) are transposed host-side, so the device does no PE transposes of
big operands and reads half the HBM bytes.

Per-core dataflow (BS = 2048 batch rows):
  F0: stream gpi_mask/gpi_w^T row tiles (two DMA queues), DVE multiplies
      mask into w^T in place -> resident mw_gpi^T [v, j] bf16; same for
      mw_gpe^T [u, i].
  F1: M[u-chunk] = sum_v mw_gpi^T[v, 1536+u-block]^T w1[v]  (PE, PSUM)
  F2: Wfold[i-chunk] = sum_v mw_gpi^T[v, i]^T w1[v]
                     + sum_u mw_gpe^T[u, i]^T M[u]
  bias fold: tiny matmuls + PE transpose of the [1, 512] row.
  B:  per 512-row tile: h1 = relu(Wfold^T x^T), h2, out -> [6, BS] f32,
      host transposes + concats.
"""

import numpy as np
import ml_dtypes

BF = ml_dtypes.bfloat16

NCORES = 8
B = 16384
BS = B // NCORES          # 2048 rows per core
BT = 512                  # batch tile (matmul free dim)
NBT = BS // BT            # 4
D1 = 1536                 # gpe input dim (x features)
D3 = 3072                 # gpi input dim
H = 512                   # mlp hidden
A = 6                     # action dim

NI = D1 // 128            # 12 i-chunks (x features)
NU = D1 // 128            # 12 u-chunks (gpe outputs)
NV = D1 // 128            # 12 v-chunks (gpi outputs)
NH = H // 128             # 4 h-chunks (mlp hidden)

_CACHE = {}


def _build():
    import concourse.bacc as bacc
    import concourse.tile as tile
    from concourse import mybir
    from concourse.masks import make_identity

    FP32 = mybir.dt.float32
    BF16 = mybir.dt.bfloat16
    Act = mybir.ActivationFunctionType

    nc = bacc.Bacc(None)

    xT_d = nc.dram_tensor("xT", [D1, BS], BF16, kind="ExternalInput")
    gpem_d = nc.dram_tensor("gpem", [D1, D1], BF16, kind="ExternalInput")
    gpewT_d = nc.dram_tensor("gpewT", [D1, D1], BF16, kind="ExternalInput")
    gpim_d = nc.dram_tensor("gpim", [D1, D3], BF16, kind="ExternalInput")
    gpiwT_d = nc.dram_tensor("gpiwT", [D1, D3], BF16, kind="ExternalInput")
    w1_d = nc.dram_tensor("w1", [D1, H], BF16, kind="ExternalInput")
    w2_d = nc.dram_tensor("w2", [H, H], BF16, kind="ExternalInput")
    w3_d = nc.dram_tensor("w3", [H, A], BF16, kind="ExternalInput")
    gpeb_d = nc.dram_tensor("gpe_b", [D1], FP32, kind="ExternalInput")
    gpib_d = nc.dram_tensor("gpi_b", [D1], FP32, kind="ExternalInput")
    b1_d = nc.dram_tensor("b1", [H], FP32, kind="ExternalInput")
    b2_d = nc.dram_tensor("b2", [H], FP32, kind="ExternalInput")
    b3_d = nc.dram_tensor("b3", [A], FP32, kind="ExternalInput")
    o_d = nc.dram_tensor("out", [A, BS], FP32, kind="ExternalOutput")

    with tile.TileContext(nc) as tc:
        with (
            tc.tile_pool(name="wp", bufs=1) as wp,           # persistent
            tc.tile_pool(name="tp", bufs=2) as tp,           # wT transients
            tc.tile_pool(name="xp", bufs=2) as xp,           # x tiles
            tc.tile_pool(name="ap", bufs=1) as ap,           # activations
            tc.tile_pool(name="psp", bufs=3, space="PSUM") as psp,
            tc.tile_pool(name="psb", bufs=1, space="PSUM") as psbp,
            tc.tile_pool(name="pso", bufs=2, space="PSUM") as psop,
            tc.tile_pool(name="pst", bufs=1, space="PSUM") as pstp,
        ):
            # ---- w1 first (gates F1), then gpi, then gpe, then the rest
            w1t = []
            for v in range(NV):
                t = wp.tile([128, H], BF16, tag=f"w1_{v}")
                nc.sync.dma_start(out=t[:, :], in_=w1_d[v * 128:(v + 1) * 128, :])
                w1t.append(t)

            # masked gpi weights, transposed layout: mwgpi[v][:, j] over j in 3072
            mwgpi = []
            for v in range(NV):
                m = wp.tile([128, D3], BF16, tag=f"mwgpi{v}")
                nc.sync.dma_start(out=m[:, :], in_=gpim_d[v * 128:(v + 1) * 128, :])
                wt = tp.tile([128, D3], BF16, tag="gwT")
                nc.gpsimd.dma_start(out=wt[:, :],
                                    in_=gpiwT_d[v * 128:(v + 1) * 128, :])
                nc.vector.tensor_mul(m[:, :], m[:, :], wt[:, :])
                mwgpi.append(m)

            # masked gpe weights, transposed layout: mwgpe[u][:, i] over i in 1536
            mwgpe = []
            for u in range(NU):
                m = wp.tile([128, D1], BF16, tag=f"mwgpe{u}")
                nc.sync.dma_start(out=m[:, :], in_=gpem_d[u * 128:(u + 1) * 128, :])
                wt = tp.tile([128, D1], BF16, tag="ewT")
                nc.gpsimd.dma_start(out=wt[:, :],
                                    in_=gpewT_d[u * 128:(u + 1) * 128, :])
                nc.vector.tensor_mul(m[:, :], m[:, :], wt[:, :])
                mwgpe.append(m)

            # ---- small stuff: w2, w3, biases
            w2t = []
            for k in range(NH):
                t = wp.tile([128, H], BF16, tag=f"w2_{k}")
                nc.sync.dma_start(out=t[:, :], in_=w2_d[k * 128:(k + 1) * 128, :])
                w2t.append(t)
            w3t = []
            for k in range(NH):
                t = wp.tile([128, A], BF16, tag=f"w3_{k}")
                nc.sync.dma_start(out=t[:, :], in_=w3_d[k * 128:(k + 1) * 128, :])
                w3t.append(t)

            ident = wp.tile([128, 128], FP32, tag="ident")
            make_identity(nc, ident[:, :])

            def load_bias_cols(b_dram, n, tag):
                """[n*128] dram -> [128, n] sbuf f32 (col c = chunk c)."""
                nat = wp.tile([n, 128], FP32, tag=f"{tag}_nat")
                nc.sync.dma_start(out=nat[:, :],
                                  in_=b_dram.rearrange("(c p) -> c p", p=128))
                ps = pstp.tile([128, n], FP32, tag="pst")
                nc.tensor.transpose(ps[:, :], nat[:, :], ident[0:n, 0:n])
                sb = wp.tile([128, n], FP32, tag=tag)
                nc.vector.tensor_copy(sb[:, :], ps[:, :])
                return sb

            gpeb_sb = load_bias_cols(gpeb_d, NU, "gpeb")
            gpib_sb = load_bias_cols(gpib_d, NV, "gpib")
            b2_sb = load_bias_cols(b2_d, NH, "b2sb")
            gpeb_bf = wp.tile([128, NU], BF16, tag="gpebf")
            nc.vector.tensor_copy(gpeb_bf[:, :], gpeb_sb[:, :])
            gpib_bf = wp.tile([128, NV], BF16, tag="gpibf")
            nc.vector.tensor_copy(gpib_bf[:, :], gpib_sb[:, :])
            b1row = wp.tile([1, H], FP32, tag="b1row")
            nc.sync.dma_start(out=b1row[:, :],
                              in_=b1_d.rearrange("(one h) -> one h", one=1))
            b3_sb = wp.tile([A, 1], FP32, tag="b3sb")
            nc.sync.dma_start(out=b3_sb[:, :],
                              in_=b3_d.rearrange("(a one) -> a one", one=1))

            # ---- x tiles stream during the fold (needed only at phase B)
            xt = [[None] * NI for _ in range(NBT)]
            for t_i in range(NBT):
                for i in range(NI):
                    t = xp.tile([128, BT], BF16, tag=f"xt{i}")
                    q = nc.gpsimd if (i % 2) else nc.sync
                    q.dma_start(out=t[:, :],
                                in_=xT_d[i * 128:(i + 1) * 128,
                                         t_i * BT:(t_i + 1) * BT])
                    xt[t_i][i] = t

            # ---- F1: M[u] = sum_v mw_gpi_g^T-block @ w1  -> bf16
            Mt = []
            for u in range(NU):
                ps = psp.tile([128, H], FP32, tag="ps")
                for v in range(NV):
                    nc.tensor.matmul(ps[:, :],
                                     mwgpi[v][:, (NI + u) * 128:(NI + u + 1) * 128],
                                     w1t[v][:, :],
                                     start=(v == 0), stop=(v == NV - 1))
                m = wp.tile([128, H], BF16, tag=f"M{u}")
                nc.scalar.activation(m[:, :], ps[:, :], Act.Copy)
                Mt.append(m)

            # ---- F2: Wfold[i] = sum_v gpi-x-part + sum_u mw_gpe^T-block @ M
            Wf = []
            for i in range(NI):
                ps = psp.tile([128, H], FP32, tag="ps")
                for v in range(NV):
                    nc.tensor.matmul(ps[:, :],
                                     mwgpi[v][:, i * 128:(i + 1) * 128],
                                     w1t[v][:, :],
                                     start=(v == 0), stop=False)
                for u in range(NU):
                    nc.tensor.matmul(ps[:, :],
                                     mwgpe[u][:, i * 128:(i + 1) * 128],
                                     Mt[u][:, :],
                                     start=False, stop=(u == NU - 1))
                w = wp.tile([128, H], BF16, tag=f"Wf{i}")
                nc.scalar.activation(w[:, :], ps[:, :], Act.Copy)
                Wf.append(w)

            # ---- bias fold: bfold = gpe_b @ M + gpi_b @ w1 + b1 -> [128, 4]
            psb = psbp.tile([1, H], FP32, tag="psb")
            for v in range(NV):
                nc.tensor.matmul(psb[:, :], gpib_bf[:, v:v + 1], w1t[v][:, :],
                                 start=(v == 0), stop=False)
            for u in range(NU):
                nc.tensor.matmul(psb[:, :], gpeb_bf[:, u:u + 1], Mt[u][:, :],
                                 start=False, stop=(u == NU - 1))
            brow = wp.tile([1, H], FP32, tag="brow")
            nc.vector.tensor_add(brow[:, :], psb[:, :], b1row[:, :])
            bfold = wp.tile([128, NH], FP32, tag="bfold")
            for c in range(NH):
                ps = pstp.tile([128, 1], FP32, tag="pstc")
                nc.tensor.transpose(ps[:, :], brow[0:1, c * 128:(c + 1) * 128],
                                    ident[0:1, 0:1])
                nc.scalar.activation(bfold[:, c:c + 1], ps[:, :], Act.Copy)

            # ---- B: batch pass over 4 tiles of 512 rows
            for t_i in range(NBT):
                h1 = []
                for hc in range(NH):
                    ps = psp.tile([128, BT], FP32, tag="ps")
                    for i in range(NI):
                        nc.tensor.matmul(ps[:, :],
                                         Wf[i][:, hc * 128:(hc + 1) * 128],
                                         xt[t_i][i][:, :],
                                         start=(i == 0), stop=(i == NI - 1))
                    h = ap.tile([128, BT], BF16, tag=f"h1_{hc}")
                    nc.scalar.activation(h[:, :], ps[:, :], Act.Relu,
                                         bias=bfold[:, hc:hc + 1])
                    h1.append(h)

                h2 = []
                for mc in range(NH):
                    ps = psp.tile([128, BT], FP32, tag="ps")
                    for k in range(NH):
                        nc.tensor.matmul(ps[:, :],
                                         w2t[k][:, mc * 128:(mc + 1) * 128],
                                         h1[k][:, :],
                                         start=(k == 0), stop=(k == NH - 1))
                    h = ap.tile([128, BT], BF16, tag=f"h2_{mc}")
                    nc.scalar.activation(h[:, :], ps[:, :], Act.Relu,
                                         bias=b2_sb[:, mc:mc + 1])
                    h2.append(h)

                pso = psop.tile([A, BT], FP32, tag="pso")
                for k in range(NH):
                    nc.tensor.matmul(pso[:, :], w3t[k][:, :], h2[k][:, :],
                                     start=(k == 0), stop=(k == NH - 1))
                osb = ap.tile([A, BT], FP32, tag="osb")
                nc.scalar.activation(osb[:, :], pso[:, :], Act.Relu,
                                     bias=b3_sb[:, 0:1])
                nc.sync.dma_start(out=o_d[:, t_i * BT:(t_i + 1) * BT],
                                  in_=osb[:, :])

    nc.finalize()
    return nc


def _get_nc():
    if "nc" not in _CACHE:
        _CACHE["nc"] = _build()
    return _CACHE["nc"]


def _prep_inputs(inputs):
    """Host-side layout/dtype prep only (no network FLOPs): bf16 casts and
    transposes so the device streams operands in the layout the PE needs."""
    f = {k: np.asarray(v) for k, v in inputs.items()}
    xT = np.ascontiguousarray(f["x"].astype(BF).T)            # [1536, B]
    shared = {
        "gpem": np.ascontiguousarray(f["gpe_mask"].astype(BF)),
        "gpewT": np.ascontiguousarray(f["gpe_w"].astype(BF).T),
        "gpim": np.ascontiguousarray(f["gpi_mask"].astype(BF)),
        "gpiwT": np.ascontiguousarray(f["gpi_w"].astype(BF).T),
        "w1": np.ascontiguousarray(f["w1"].astype(BF)),
        "w2": np.ascontiguousarray(f["w2"].astype(BF)),
        "w3": np.ascontiguousarray(f["w3"].astype(BF)),
        "gpe_b": np.ascontiguousarray(f["gpe_b"], dtype=np.float32),
        "gpi_b": np.ascontiguousarray(f["gpi_b"], dtype=np.float32),
        "b1": np.ascontiguousarray(f["b1"], dtype=np.float32),
        "b2": np.ascontiguousarray(f["b2"], dtype=np.float32),
        "b3": np.ascontiguousarray(f["b3"], dtype=np.float32),
    }
    in_maps = [
        dict(shared, xT=np.ascontiguousarray(xT[:, c * BS:(c + 1) * BS]))
        for c in range(NCORES)
    ]
    return in_maps


def _run(inputs, trace=False):
    from concourse.bass_utils import run_bass_kernel_spmd

    nc = _get_nc()
    in_maps = _prep_inputs(inputs)
    res = run_bass_kernel_spmd(nc, in_maps, list(range(NCORES)), trace=trace)
    out = np.concatenate(
        [np.asarray(res.results[c]["out"]).T for c in range(NCORES)], axis=0)
    return out.astype(np.float32), res


def kernel(**inputs):
    out, _ = _run(inputs, trace=False)
    return out


# revision 7
# speedup vs baseline: 3.1685x; 1.2502x over previous
"""CTBG circuit kernel for Trainium2, data-parallel over batch on 8 NeuronCores.

Network (per reference):
  gpe_out = x @ (gpe_w * gpe_mask.T) + gpe_b              [B, 1536]
  gpi_in  = concat([x, gpe_out], -1)                      [B, 3072]
  gpi_out = gpi_in @ (gpi_w * gpi_mask.T) + gpi_b         [B, 1536]
  h1 = relu(gpi_out @ w1 + b1); h2 = relu(h1 @ w2 + b2)
  out = relu(h2 @ w3 + b3)                                [B, 6]

Key algebraic identity: gpe_out and gpi_out feed forward with no
intervening nonlinearity, so the masked front end folds into one
[1536, 512] weight computed ON DEVICE once per launch:

  mw_gpe = gpe_w * gpe_mask.T
  mw_gpi = gpi_w * gpi_mask.T
  M      = mw_gpi[1536:] @ w1                       [1536, 512]
  Wfold  = mw_gpi[:1536] @ w1 + mw_gpe @ M          [1536, 512]
  bfold  = gpe_b @ M + gpi_b @ w1 + b1              [512]
  h1 = relu(x @ Wfold + bfold)   -> h2 -> out       (per batch row)

The fold itself is SHARDED across the 8 cores: core c computes rows
[c*192, (c+1)*192) of M (then of Wfold), which takes only the
corresponding COLUMN slices of the masks/weights (sliced host-side, a
pure layout op) — so each core streams ~5 MB of fold operands instead
of ~28 MB, and does 1/8 of the fold matmuls. Slices are assembled with
two DRAM AllGathers (M, then Wfold).

Host prep is layout/dtype only (no FLOPs): bf16 casts, transposes of
x/gpe_w/gpi_w, and column slicing.

Per-core phases (BS = 2048 batch rows):
  F0:  stream sliced mask columns + w^T columns, DVE-multiply in place.
  F1s: M_slice = sum_v mwgpiT[v, uslice]^T @ w1[v]  -> DRAM, AllGather.
  F2s: Wf_slice = sum_v mwgpiT[v, islice]^T w1[v]
                + sum_u mwgpeT[u, islice]^T M[u]    -> DRAM, AllGather.
  bias fold: tiny matmuls on gathered M + PE transpose of [1, 512] row.
  B:   per 512-row tile: h1 = relu(Wfold^T x^T), h2, out -> [6, BS] f32;
       host transposes + concats.
"""

import numpy as np
import ml_dtypes

BF = ml_dtypes.bfloat16

NCORES = 8
B = 16384
BS = B // NCORES          # 2048 rows per core
BT = 512                  # batch tile (matmul free dim)
NBT = BS // BT            # 4
D1 = 1536                 # gpe input dim (x features)
D3 = 3072                 # gpi input dim
H = 512                   # mlp hidden
A = 6                     # action dim
SL = D1 // NCORES         # 192: fold rows per core

NI = D1 // 128            # 12 i-chunks (x features)
NU = D1 // 128            # 12 u-chunks (gpe outputs)
NV = D1 // 128            # 12 v-chunks (gpi outputs)
NH = H // 128             # 4 h-chunks (mlp hidden)

_CACHE = {}


def _build():
    import concourse.bacc as bacc
    import concourse.tile as tile
    from concourse import mybir
    from concourse.masks import make_identity

    FP32 = mybir.dt.float32
    BF16 = mybir.dt.bfloat16
    Act = mybir.ActivationFunctionType

    nc = bacc.Bacc(None, num_devices=NCORES)

    xT_d = nc.dram_tensor("xT", [D1, BS], BF16, kind="ExternalInput")
    # column slices for this core's fold rows: gpi gets [islice | uslice]
    # (384 cols), gpe gets [islice] (192 cols)
    gpims_d = nc.dram_tensor("gpims", [D1, 2 * SL], BF16, kind="ExternalInput")
    gpiwTs_d = nc.dram_tensor("gpiwTs", [D1, 2 * SL], BF16, kind="ExternalInput")
    gpems_d = nc.dram_tensor("gpems", [D1, SL], BF16, kind="ExternalInput")
    gpewTs_d = nc.dram_tensor("gpewTs", [D1, SL], BF16, kind="ExternalInput")
    w1_d = nc.dram_tensor("w1", [D1, H], BF16, kind="ExternalInput")
    w2_d = nc.dram_tensor("w2", [H, H], BF16, kind="ExternalInput")
    w3_d = nc.dram_tensor("w3", [H, A], BF16, kind="ExternalInput")
    gpeb_d = nc.dram_tensor("gpe_b", [D1], FP32, kind="ExternalInput")
    gpib_d = nc.dram_tensor("gpi_b", [D1], FP32, kind="ExternalInput")
    b1_d = nc.dram_tensor("b1", [H], FP32, kind="ExternalInput")
    b2_d = nc.dram_tensor("b2", [H], FP32, kind="ExternalInput")
    b3_d = nc.dram_tensor("b3", [A], FP32, kind="ExternalInput")
    o_d = nc.dram_tensor("out", [A, BS], FP32, kind="ExternalOutput")

    RG = [list(range(NCORES))]

    with tile.TileContext(nc) as tc:
        with (
            tc.tile_pool(name="wp", bufs=1) as wp,           # persistent
            tc.tile_pool(name="tp", bufs=2) as tp,           # wT transients
            tc.tile_pool(name="xp", bufs=3) as xp,           # x tiles
            tc.tile_pool(name="ap", bufs=1) as ap,           # activations
            tc.tile_pool(name="dp", bufs=1, space="DRAM") as dp,
            tc.tile_pool(name="psp", bufs=3, space="PSUM") as psp,
            tc.tile_pool(name="ps2", bufs=1, space="PSUM") as ps2p,
            tc.tile_pool(name="pso", bufs=2, space="PSUM") as psop,
            tc.tile_pool(name="pst", bufs=1, space="PSUM") as pstp,
        ):
            # ---- w1 first (gates F1s), then sliced gpi, gpe
            w1t = []
            for v in range(NV):
                t = wp.tile([128, H], BF16, tag=f"w1_{v}")
                nc.sync.dma_start(out=t[:, :], in_=w1_d[v * 128:(v + 1) * 128, :])
                w1t.append(t)

            # masked gpi columns, [v-part, 384]: cols 0:192 = islice,
            # 192:384 = uslice
            mwgpi = []
            for v in range(NV):
                m = wp.tile([128, 2 * SL], BF16, tag=f"mwgpi{v}")
                nc.sync.dma_start(out=m[:, :], in_=gpims_d[v * 128:(v + 1) * 128, :])
                wt = tp.tile([128, 2 * SL], BF16, tag="gwT")
                nc.gpsimd.dma_start(out=wt[:, :],
                                    in_=gpiwTs_d[v * 128:(v + 1) * 128, :])
                nc.vector.tensor_mul(m[:, :], m[:, :], wt[:, :])
                mwgpi.append(m)

            # masked gpe columns, [u-part, 192]: cols = islice
            mwgpe = []
            for u in range(NU):
                m = wp.tile([128, SL], BF16, tag=f"mwgpe{u}")
                nc.sync.dma_start(out=m[:, :], in_=gpems_d[u * 128:(u + 1) * 128, :])
                wt = tp.tile([128, SL], BF16, tag="ewT")
                nc.gpsimd.dma_start(out=wt[:, :],
                                    in_=gpewTs_d[u * 128:(u + 1) * 128, :])
                nc.vector.tensor_mul(m[:, :], m[:, :], wt[:, :])
                mwgpe.append(m)

            # ---- small stuff: w2, w3, biases
            w2t = []
            for k in range(NH):
                t = wp.tile([128, H], BF16, tag=f"w2_{k}")
                nc.sync.dma_start(out=t[:, :], in_=w2_d[k * 128:(k + 1) * 128, :])
                w2t.append(t)
            w3t = []
            for k in range(NH):
                t = wp.tile([128, A], BF16, tag=f"w3_{k}")
                nc.sync.dma_start(out=t[:, :], in_=w3_d[k * 128:(k + 1) * 128, :])
                w3t.append(t)

            ident = wp.tile([128, 128], FP32, tag="ident")
            make_identity(nc, ident[:, :])

            def load_bias_cols(b_dram, n, tag):
                nat = wp.tile([n, 128], FP32, tag=f"{tag}_nat")
                nc.sync.dma_start(out=nat[:, :],
                                  in_=b_dram.rearrange("(c p) -> c p", p=128))
                ps = pstp.tile([128, n], FP32, tag="pst")
                nc.tensor.transpose(ps[:, :], nat[:, :], ident[0:n, 0:n])
                sb = wp.tile([128, n], FP32, tag=tag)
                nc.vector.tensor_copy(sb[:, :], ps[:, :])
                return sb

            gpeb_sb = load_bias_cols(gpeb_d, NU, "gpeb")
            gpib_sb = load_bias_cols(gpib_d, NV, "gpib")
            b2_sb = load_bias_cols(b2_d, NH, "b2sb")
            gpeb_bf = wp.tile([128, NU], BF16, tag="gpebf")
            nc.vector.tensor_copy(gpeb_bf[:, :], gpeb_sb[:, :])
            gpib_bf = wp.tile([128, NV], BF16, tag="gpibf")
            nc.vector.tensor_copy(gpib_bf[:, :], gpib_sb[:, :])
            b1row = wp.tile([1, H], FP32, tag="b1row")
            nc.sync.dma_start(out=b1row[:, :],
                              in_=b1_d.rearrange("(one h) -> one h", one=1))
            b3_sb = wp.tile([A, 1], FP32, tag="b3sb")
            nc.sync.dma_start(out=b3_sb[:, :],
                              in_=b3_d.rearrange("(a one) -> a one", one=1))

            # ---- x tiles stream in the background
            xt = [[None] * NI for _ in range(NBT)]
            for t_i in range(NBT):
                for i in range(NI):
                    t = xp.tile([128, BT], BF16, tag=f"xt{i}")
                    q = nc.gpsimd if (i % 2) else nc.sync
                    q.dma_start(out=t[:, :],
                                in_=xT_d[i * 128:(i + 1) * 128,
                                         t_i * BT:(t_i + 1) * BT])
                    xt[t_i][i] = t

            # ---- F1s: M_slice[r, h] = sum_v mwgpiT[v, 1536+uslice][r] w1[v]
            # slice rows split as 128 + 64
            msl_dram = dp.tile([SL, H], BF16, tag="msl_d")
            for g, (r0, rn) in enumerate([(0, 128), (128, SL - 128)]):
                ps = psp.tile([128, H], FP32, tag="ps")
                for v in range(NV):
                    nc.tensor.matmul(ps[0:rn, :],
                                     mwgpi[v][:, SL + r0:SL + r0 + rn],
                                     w1t[v][:, :],
                                     start=(v == 0), stop=(v == NV - 1))
                sb = wp.tile([128, H], BF16, tag=f"mslice{g}")
                nc.scalar.activation(sb[0:rn, :], ps[0:rn, :], Act.Copy)
                nc.sync.dma_start(out=msl_dram[r0:r0 + rn, :], in_=sb[0:rn, :])
            mfull_dram = dp.tile([D1, H], BF16, tag="mfull_d")
            nc.gpsimd.collective_compute(
                "AllGather", mybir.AluOpType.bypass, replica_groups=RG,
                ins=[msl_dram[:, :].opt()], outs=[mfull_dram[:, :].opt()])
            Mt = []
            for u in range(NU):
                t = wp.tile([128, H], BF16, tag=f"M{u}")
                q = nc.gpsimd if (u % 2) else nc.sync
                q.dma_start(out=t[:, :], in_=mfull_dram[u * 128:(u + 1) * 128, :])
                Mt.append(t)

            # ---- F2s: Wf_slice = gpi-x-part + mwgpe-slice^T @ M
            wfs_dram = dp.tile([SL, H], BF16, tag="wfs_d")
            for g, (r0, rn) in enumerate([(0, 128), (128, SL - 128)]):
                ps = psp.tile([128, H], FP32, tag="ps")
                for v in range(NV):
                    nc.tensor.matmul(ps[0:rn, :],
                                     mwgpi[v][:, r0:r0 + rn],
                                     w1t[v][:, :],
                                     start=(v == 0), stop=False)
                for u in range(NU):
                    nc.tensor.matmul(ps[0:rn, :],
                                     mwgpe[u][:, r0:r0 + rn],
                                     Mt[u][:, :],
                                     start=False, stop=(u == NU - 1))
                sb = wp.tile([128, H], BF16, tag=f"wfslice{g}")
                nc.scalar.activation(sb[0:rn, :], ps[0:rn, :], Act.Copy)
                nc.sync.dma_start(out=wfs_dram[r0:r0 + rn, :], in_=sb[0:rn, :])
            wff_dram = dp.tile([D1, H], BF16, tag="wff_d")
            nc.gpsimd.collective_compute(
                "AllGather", mybir.AluOpType.bypass, replica_groups=RG,
                ins=[wfs_dram[:, :].opt()], outs=[wff_dram[:, :].opt()])
            Wf = []
            for i in range(NI):
                t = wp.tile([128, H], BF16, tag=f"Wf{i}")
                q = nc.gpsimd if (i % 2) else nc.sync
                q.dma_start(out=t[:, :], in_=wff_dram[i * 128:(i + 1) * 128, :])
                Wf.append(t)

            # ---- bias fold: bfold = gpe_b @ M + gpi_b @ w1 + b1 -> [128, 4]
            psb = ps2p.tile([1, H], FP32, tag="psb")
            for v in range(NV):
                nc.tensor.matmul(psb[:, :], gpib_bf[:, v:v + 1], w1t[v][:, :],
                                 start=(v == 0), stop=False)
            for u in range(NU):
                nc.tensor.matmul(psb[:, :], gpeb_bf[:, u:u + 1], Mt[u][:, :],
                                 start=False, stop=(u == NU - 1))
            brow = wp.tile([1, H], FP32, tag="brow")
            nc.vector.tensor_add(brow[:, :], psb[:, :], b1row[:, :])
            bfold = wp.tile([128, NH], FP32, tag="bfold")
            for c in range(NH):
                ps = pstp.tile([128, 1], FP32, tag="pstc")
                nc.tensor.transpose(ps[:, :], brow[0:1, c * 128:(c + 1) * 128],
                                    ident[0:1, 0:1])
                nc.scalar.activation(bfold[:, c:c + 1], ps[:, :], Act.Copy)

            # ---- B: batch pass over 4 tiles of 512 rows
            for t_i in range(NBT):
                h1 = []
                for hc in range(NH):
                    ps = psp.tile([128, BT], FP32, tag="ps")
                    for i in range(NI):
                        nc.tensor.matmul(ps[:, :],
                                         Wf[i][:, hc * 128:(hc + 1) * 128],
                                         xt[t_i][i][:, :],
                                         start=(i == 0), stop=(i == NI - 1))
                    h = ap.tile([128, BT], BF16, tag=f"h1_{hc}")
                    nc.scalar.activation(h[:, :], ps[:, :], Act.Relu,
                                         bias=bfold[:, hc:hc + 1])
                    h1.append(h)

                h2 = []
                for mc in range(NH):
                    ps = psp.tile([128, BT], FP32, tag="ps")
                    for k in range(NH):
                        nc.tensor.matmul(ps[:, :],
                                         w2t[k][:, mc * 128:(mc + 1) * 128],
                                         h1[k][:, :],
                                         start=(k == 0), stop=(k == NH - 1))
                    h = ap.tile([128, BT], BF16, tag=f"h2_{mc}")
                    nc.scalar.activation(h[:, :], ps[:, :], Act.Relu,
                                         bias=b2_sb[:, mc:mc + 1])
                    h2.append(h)

                pso = psop.tile([A, BT], FP32, tag="pso")
                for k in range(NH):
                    nc.tensor.matmul(pso[:, :], w3t[k][:, :], h2[k][:, :],
                                     start=(k == 0), stop=(k == NH - 1))
                osb = ap.tile([A, BT], FP32, tag="osb")
                nc.scalar.activation(osb[:, :], pso[:, :], Act.Relu,
                                     bias=b3_sb[:, 0:1])
                nc.sync.dma_start(out=o_d[:, t_i * BT:(t_i + 1) * BT],
                                  in_=osb[:, :])

    nc.finalize()
    return nc


def _get_nc():
    if "nc" not in _CACHE:
        _CACHE["nc"] = _build()
    return _CACHE["nc"]


def _prep_inputs(inputs):
    """Host-side layout/dtype prep only (no network FLOPs): bf16 casts,
    transposes, and per-core column slicing of the fold operands."""
    f = {k: np.asarray(v) for k, v in inputs.items()}
    xT = np.ascontiguousarray(f["x"].astype(BF).T)            # [1536, B]
    gpem = f["gpe_mask"].astype(BF)                           # [u, i]
    gpewT = np.ascontiguousarray(f["gpe_w"].astype(BF).T)     # [u, i]
    gpim = f["gpi_mask"].astype(BF)                           # [v, j]
    gpiwT = np.ascontiguousarray(f["gpi_w"].astype(BF).T)     # [v, j]
    shared = {
        "w1": np.ascontiguousarray(f["w1"].astype(BF)),
        "w2": np.ascontiguousarray(f["w2"].astype(BF)),
        "w3": np.ascontiguousarray(f["w3"].astype(BF)),
        "gpe_b": np.ascontiguousarray(f["gpe_b"], dtype=np.float32),
        "gpi_b": np.ascontiguousarray(f["gpi_b"], dtype=np.float32),
        "b1": np.ascontiguousarray(f["b1"], dtype=np.float32),
        "b2": np.ascontiguousarray(f["b2"], dtype=np.float32),
        "b3": np.ascontiguousarray(f["b3"], dtype=np.float32),
    }
    in_maps = []
    for c in range(NCORES):
        isl = slice(c * SL, (c + 1) * SL)
        usl = slice(D1 + c * SL, D1 + (c + 1) * SL)
        in_maps.append(dict(
            shared,
            xT=np.ascontiguousarray(xT[:, c * BS:(c + 1) * BS]),
            gpims=np.ascontiguousarray(
                np.concatenate([gpim[:, isl], gpim[:, usl]], axis=1)),
            gpiwTs=np.ascontiguousarray(
                np.concatenate([gpiwT[:, isl], gpiwT[:, usl]], axis=1)),
            gpems=np.ascontiguousarray(gpem[:, isl]),
            gpewTs=np.ascontiguousarray(gpewT[:, isl]),
        ))
    return in_maps


def _run(inputs, trace=False):
    from concourse.bass_utils import run_bass_kernel_spmd

    nc = _get_nc()
    in_maps = _prep_inputs(inputs)
    res = run_bass_kernel_spmd(nc, in_maps, list(range(NCORES)), trace=trace)
    out = np.concatenate(
        [np.asarray(res.results[c]["out"]).T for c in range(NCORES)], axis=0)
    return out.astype(np.float32), res


def kernel(**inputs):
    out, _ = _run(inputs, trace=False)
    return out
